# revision 1
# baseline (speedup 1.0000x reference)
"""Trainium2 Bass kernel for nn_NeuralNetwork_86990267613505 (topk_masking).

Network (per reference):
  cx = sigmoid(tanh(input @ W_c1.T + b_c1) @ W_c2.T)          # [B] gate
  x  = kwta(input @ W1.T + b1, k=int(cx*1024))                # [B,1024]
  x  = kwta(x @ W2.T + b2,     k=int(cx*512))                 # [B,512]
  x  = kwta(x @ W3.T + b3,     k=int(cx*1024))                # [B,1024]
  out = x @ W4.T                                              # [B,1024]

Sharding: the two big matmuls (contraction over S2=32768) are column-sharded
over the contraction dim across 8 cores (4096 each); partial sums are combined
with a single fused ReduceScatter of [B, 512+1024] which also distributes the
batch (32 rows per core).  Everything after is data-parallel per core.

kwta: per-row exact k-th-largest via radix-5 bisection (probes replicated 4x
across partitions), then band extraction + two max8 passes + indicator-pick,
then mask = (x >= thresh) * x.

Matmuls run as float32r (full-speed fp32 container with ~12-bit multiply
mantissa).
"""

import numpy as np

import concourse.bacc as bacc
import concourse.mybir as mybir
import concourse.tile as tile
from concourse import bass_utils

F32 = mybir.dt.float32
F32R = mybir.dt.float32r
BF16 = mybir.dt.bfloat16
I32 = mybir.dt.int32
ALU = mybir.AluOpType
ACTF = mybir.ActivationFunctionType

HID = 512
N1 = 2 * HID      # 1024
N3 = 1024         # HEADS
R = 32            # rows per core after scatter
C = 4             # partition replication for probing
BIG = 1e30
N_PASS = 6        # radix-5 bisection passes


class Cfg:
    def __init__(self, S2=32768, B=256, NC=8, chunk=4, debug=False):
        assert B // NC == R
        self.S2, self.B, self.NC, self.chunk = S2, B, NC, chunk
        self.debug = debug
        self.no_collective = False
        self.loop_n = 0
        self.phase = None  # None | 'notail'
        self.KSH = S2 // NC            # contraction shard per core
        self.KT = self.KSH // 128      # k-tiles
        assert self.KT % chunk == 0
        self.SW = B + 3 * HID          # stream free width per k-tile
        # b-tiles: chunks of <=128 rows of the full batch
        self.b_tiles = [(s, min(128, B - s)) for s in range(0, B, 128)]


def _floorize(nc, sb, val_ap, name):
    """floor(val) for val >= 0, given HW float->int casts are RNE."""
    ki = sb.tile([128, 1], I32, name=f"{name}_i")
    kb = sb.tile([128, 1], F32, name=f"{name}_b")
    cmp = sb.tile([128, 1], F32, name=f"{name}_c")
    kf = sb.tile([128, 1], F32, name=f"{name}_f")
    nc.vector.tensor_copy(ki[:], val_ap)
    nc.vector.tensor_copy(kb[:], ki[:])
    nc.vector.tensor_tensor(cmp[:], kb[:], val_ap, ALU.is_gt)
    nc.vector.tensor_sub(kf[:], kb[:], cmp[:])
    return kf


def _pe_keepalive(nc, ps, src_ap, lname, i):
    pdum = ps.tile([1, 1], F32, tag="tp", bufs=2, name=f"{lname}_pd{i}")
    nc.tensor.matmul(pdum[:], src_ap, src_ap, start=True, stop=True)


def _kwta(nc, sb, ps, x_ap, krepf, n, consts, lname):
    """x_ap: [128, n] fp32 SBUF (rows replicated 4x: partition 32c+r = row r).
    krepf: [128,1] fp32 float(k).  Returns masked [R, n] f32r tile."""
    frac, iota16, ident = consts["frac"], consts["iota16"], consts["ident"]

    lo = sb.tile([128, 1], F32, tag="kw_lo", bufs=2, name=f"{lname}_lo0")
    width = sb.tile([128, 1], F32, name=f"{lname}_w")
    probes = sb.tile([128, 1], F32, tag="kw_pr", bufs=2, name=f"{lname}_pr0")
    nc.vector.memset(lo[:], -16.0)
    nc.vector.memset(width[:], 32.0)
    # probes = frac*32 - 16
    nc.vector.tensor_scalar(probes[:], frac[:], 32.0, -16.0, ALU.mult, ALU.add)

    xb = sb.tile([128, n], BF16, tag="kw_xb", name=f"{lname}_xb0")
    nc.vector.tensor_copy(xb[:], x_ap)
    trash = sb.tile([128, n], BF16, tag="kw_tr", name=f"{lname}_tr0")
    cnt = sb.tile([128, 1], F32, tag="kw_cnt", bufs=2, name=f"{lname}_cnt0")
    for p in range(N_PASS):
        nc.vector.tensor_scalar(
            trash[:], xb[:], probes[:, 0:1], None, ALU.is_ge, ALU.add,
            accum_out=cnt[:],
        )
        ge = sb.tile([128, 1], F32, tag="kw_ge", bufs=2, name=f"{lname}_ge{p}")
        nc.vector.tensor_scalar(ge[:], cnt[:], krepf[:, 0:1], None, ALU.is_ge)
        sh64 = sb.tile([64, 1], F32, tag="kw_s64", bufs=2, name=f"{lname}_s64_{p}")
        f2 = sb.tile([64, 1], F32, tag="kw_f2", bufs=2, name=f"{lname}_f2_{p}")
        sh32 = sb.tile([32, 1], F32, tag="kw_s32", bufs=2, name=f"{lname}_s32_{p}")
        jall = sb.tile([128, 1], F32, tag="kw_j", bufs=2, name=f"{lname}_j{p}")
        nc.vector.tensor_copy(sh64[:], ge[64:128, :])
        nc.vector.tensor_add(f2[:], ge[0:64, :], sh64[:])
        nc.vector.tensor_copy(sh32[:], f2[32:64, :])
        nc.vector.tensor_add(jall[0:32, :], f2[0:32, :], sh32[:])
        nc.vector.tensor_copy(jall[32:64, :], jall[0:32, :])
        nc.vector.tensor_copy(jall[64:128, :], jall[0:64, :])
        # width /= 5 ; lo += width_new * j ; probes = frac*width_new + lo_new
        nc.vector.tensor_scalar(width[:], width[:], 0.2, None, ALU.mult)
        lo_new = sb.tile([128, 1], F32, tag="kw_lo", bufs=2, name=f"{lname}_lo{p+1}")
        nc.scalar.activation(lo_new[:], jall[:], ACTF.Identity,
                             bias=lo[:, 0:1], scale=width[:, 0:1])
        probes_new = sb.tile([128, 1], F32, tag="kw_pr", bufs=2, name=f"{lname}_pr{p+1}")
        nc.scalar.activation(probes_new[:], frac[:], ACTF.Identity,
                             bias=lo_new[:, 0:1], scale=width[:, 0:1])
        _pe_keepalive(nc, ps, probes_new[0:1, 0:1], lname, p)
        lo, probes = lo_new, probes_new

    hi = sb.tile([128, 1], F32, name=f"{lname}_hi")
    nc.scalar.activation(hi[:], width[:], ACTF.Identity, bias=lo[:, 0:1], scale=1.0)
    # c_hi = count(x >= hi)
    chi = sb.tile([128, 1], F32, name=f"{lname}_chi")
    nc.vector.tensor_scalar(
        trash[:], xb[:], hi[:, 0:1], None, ALU.is_ge, ALU.add, accum_out=chi[:],
    )
    _pe_keepalive(nc, ps, chi[0:1, 0:1], lname, "chi")
    # band values on rows 0:R: x in [lo, hi) else -BIG
    x_r = x_ap[0:R, :]
    bhi = sb.tile([R, n], F32, tag="kw_bhi", name=f"{lname}_bhi0")
    binb = sb.tile([R, n], I32, tag="kw_binb", name=f"{lname}_binb0")
    bandv = sb.tile([R, n], F32, tag="kw_bv", name=f"{lname}_bv0")
    nc.vector.tensor_scalar(bhi[:], xb[0:R, :], hi[0:R, 0:1], None, ALU.is_lt)
    nc.vector.scalar_tensor_tensor(
        binb[:], xb[0:R, :], lo[0:R, 0:1], bhi[:], ALU.is_ge, ALU.mult)
    nc.vector.memset(bandv[:], -BIG)
    nc.vector.copy_predicated(bandv[:], binb[:], x_r)
    # top-16 of band
    m16 = sb.tile([R, 16], F32, name=f"{lname}_m16")
    band2 = sb.tile([R, n], F32, tag="kw_b2", name=f"{lname}_b20")
    nc.vector.max(m16[:, 0:8], bandv[:])
    nc.vector.match_replace(band2[:], m16[:, 0:8], bandv[:], -BIG)
    nc.vector.max(m16[:, 8:16], band2[:])
    _pe_keepalive(nc, ps, m16[0:1, 0:1], lname, "m16")
    # pick (k - c_hi - 1)-th
    rf = sb.tile([R, 1], F32, name=f"{lname}_rf")
    nc.vector.tensor_sub(rf[:], krepf[0:R, :], chi[0:R, :])
    nc.vector.tensor_scalar(rf[:], rf[:], 1.0, None, ALU.subtract)
    nc.vector.tensor_scalar(rf[:], rf[:], 0.0, 15.0, ALU.max, ALU.min)
    ind = sb.tile([R, 16], F32, name=f"{lname}_ind")
    nc.vector.tensor_scalar(ind[:], iota16[0:R, :], rf[:, 0:1], None, ALU.is_equal)
    iv = sb.tile([R, 16], F32, name=f"{lname}_iv")
    nc.vector.tensor_mul(iv[:], ind[:], m16[:])
    vk = sb.tile([R, 1], F32, name=f"{lname}_vk")
    nc.vector.reduce_sum(vk[:], iv[:], axis=mybir.AxisListType.X)
    # thresh = k>=1 ? vk : +BIG
    g = sb.tile([R, 1], F32, name=f"{lname}_g")
    ga = sb.tile([R, 1], F32, name=f"{lname}_ga")
    gb = sb.tile([R, 1], F32, name=f"{lname}_gb")
    thr = sb.tile([R, 1], F32, name=f"{lname}_thr")
    nc.vector.tensor_scalar(g[:], krepf[0:R, :], 1.0, None, ALU.is_ge)
    nc.vector.tensor_scalar(ga[:], g[:], -BIG, BIG, ALU.mult, ALU.add)
    nc.vector.tensor_scalar(gb[:], g[:], vk[:, 0:1], None, ALU.mult)
    nc.vector.tensor_add(thr[:], ga[:], gb[:])
    # masked = (x >= thresh) * x   (f32r out, feeds matmuls)
    masked = sb.tile([R, n], F32, tag="kw_mask", name=f"{lname}_masked")
    nc.vector.scalar_tensor_tensor(
        masked[:], x_r, thr[:, 0:1], x_r, ALU.is_ge, ALU.mult)
    return masked, thr


def _transpose_chunks(nc, sb, pst, masked, n, ident, rep, lname):
    """masked [R, n] f32r -> list of xT tiles: [128, C*R] (rep) or [128, R]."""
    tiles = []
    for ch in range(n // 128):
        pt = pst.tile([128, R], F32, tag="tp", name=f"{lname}_pt{ch}")
        nc.tensor.transpose(pt[:], masked[:, 128 * ch:128 * (ch + 1)],
                            ident[0:R, 0:R])
        if rep:
            xt = sb.tile([128, C * R], F32, tag="kw_xt", bufs=8,
                         name=f"{lname}_xt{ch}")
            nc.vector.tensor_copy(
                xt[:].rearrange("p (c r) -> p c r", c=C),
                pt[:, :].unsqueeze(1).broadcast_to([128, C, R]),
            )
        else:
            xt = sb.tile([128, R], F32, tag="kw_xt", bufs=8,
                         name=f"{lname}_xt{ch}")
            nc.vector.tensor_copy(xt[:], pt[:])
        tiles.append(xt)
    return tiles


def build_nc(cfg: Cfg):
    nc = bacc.Bacc("TRN2", target_bir_lowering=False, debug=False,
                   num_devices=cfg.NC)
    B, NC, KT, SW, chunk = cfg.B, cfg.NC, cfg.KT, cfg.SW, cfg.chunk

    stream_d = nc.dram_tensor("stream", [KT, 128, 2, SW], BF16, kind="ExternalInput")
    ident_d = nc.dram_tensor("ident", [128, 128], F32, kind="ExternalInput")
    biasc_d = nc.dram_tensor("biasc", [128, 3 * HID], F32, kind="ExternalInput")
    b2rep_d = nc.dram_tensor("b2rep", [128, HID], F32, kind="ExternalInput")
    b3rep_d = nc.dram_tensor("b3rep", [128, N3], F32, kind="ExternalInput")
    wc2rep_d = nc.dram_tensor("wc2rep", [128, HID], F32, kind="ExternalInput")
    frac_d = nc.dram_tensor("frac", [128, 1], F32, kind="ExternalInput")
    iota16_d = nc.dram_tensor("iota16", [R, 16], F32, kind="ExternalInput")
    w2t_d = nc.dram_tensor("w2t", [N1, HID], F32, kind="ExternalInput")
    w3t_d = nc.dram_tensor("w3t", [HID, N3], F32, kind="ExternalInput")
    w4t_d = nc.dram_tensor("w4t", [N3, N3], F32, kind="ExternalInput")
    out_d = nc.dram_tensor("out", [R, N3], F32, kind="ExternalOutput")
    if cfg.debug:
        dbg_rs_d = nc.dram_tensor("dbg_rs", [R, 3 * HID], F32, kind="ExternalOutput")
        dbg_gate_d = nc.dram_tensor("dbg_gate", [R, 8], F32, kind="ExternalOutput")
        dbg_x2_d = nc.dram_tensor("dbg_x2", [R, HID], F32, kind="ExternalOutput")
        dbg_x3_d = nc.dram_tensor("dbg_x3", [R, N3], F32, kind="ExternalOutput")

    import contextlib
    with tile.TileContext(nc) as tc:
        loop_ctx = tc.For_i(0, cfg.loop_n, 1) if cfg.loop_n else contextlib.nullcontext()
        with (
            loop_ctx,
            tc.tile_pool(name="consts", bufs=1) as cp,
            tc.tile_pool(name="stream", bufs=2) as sp,
            tc.tile_pool(name="acc", bufs=1, space="PSUM") as ap,
            tc.tile_pool(name="sb", bufs=1) as sb,
            tc.tile_pool(name="pst", bufs=2, space="PSUM") as pst,
            tc.tile_pool(name="dram", bufs=1, space="DRAM") as dram,
        ):
            # ---- constants ----
            ident = cp.tile([128, 128], F32, name="ident")
            biasc = cp.tile([128, 3 * HID], F32, name="biasc")
            b2rep = cp.tile([128, HID], F32, name="b2rep")
            b3rep = cp.tile([128, N3], F32, name="b3rep")
            wc2rep = cp.tile([128, HID], F32, name="wc2rep")
            frac = cp.tile([128, 1], F32, name="frac")
            iota16 = cp.tile([R, 16], F32, name="iota16")
            nc.sync.dma_start(ident[:], ident_d.ap())
            nc.sync.dma_start(biasc[:], biasc_d.ap())
            nc.sync.dma_start(b2rep[:], b2rep_d.ap())
            nc.sync.dma_start(b3rep[:], b3rep_d.ap())
            nc.sync.dma_start(wc2rep[:], wc2rep_d.ap())
            nc.sync.dma_start(frac[:], frac_d.ap())
            nc.sync.dma_start(iota16[:], iota16_d.ap())
            consts = {"ident": ident, "frac": frac, "iota16": iota16}

            # ---- phase A: streamed big matmuls ----
            pc1 = {}
            p1 = {}
            for bi, (bs, bsz) in enumerate(cfg.b_tiles):
                pc1[bi] = ap.tile([bsz, HID], F32, tag="acc", bufs=6, name=f"pc1_{bi}")
                p1[bi] = [ap.tile([bsz, HID], F32, tag="acc", bufs=6, name=f"p1_{bi}_{o}")
                          for o in range(2)]

            n_chunks = KT // chunk
            for cki in range(n_chunks):
                st = sp.tile([128, chunk * 2 * SW], BF16, tag="st", name=f"st{cki}")
                src = stream_d.ap()[chunk * cki: chunk * (cki + 1)]
                nc.sync.dma_start(
                    st[:].rearrange("p (c t w) -> p c t w", c=chunk, t=2),
                    src.transpose([1, 0, 2, 3]),
                )
                for ki in range(chunk):
                    kt = chunk * cki + ki
                    hi = st[:, (2 * ki) * SW:(2 * ki + 1) * SW]
                    lo = st[:, (2 * ki + 1) * SW:(2 * ki + 2) * SW]
                    first, last = kt == 0, kt == KT - 1
                    for pi, (xa, wb) in enumerate(((hi, hi), (hi, lo), (lo, hi))):
                        f = first and pi == 0
                        l = last and pi == 2
                        for bi, (bs, bsz) in enumerate(cfg.b_tiles):
                            lhsT = xa[:, bs:bs + bsz]
                            nc.tensor.matmul(pc1[bi][:], lhsT, wb[:, B:B + HID],
                                             start=f, stop=l)
                            nc.tensor.matmul(p1[bi][0][:], lhsT,
                                             wb[:, B + HID:B + 2 * HID],
                                             start=f, stop=l)
                            nc.tensor.matmul(p1[bi][1][:], lhsT,
                                             wb[:, B + 2 * HID:B + 3 * HID],
                                             start=f, stop=l)

            # ---- tail weights (after the stream so they don't delay it) ----
            w2sb = cp.tile([128, 8 * HID], F32, name="w2sb")
            w3sb = cp.tile([128, 4 * N3], F32, name="w3sb")
            w4sb = cp.tile([128, 8 * N3], F32, name="w4sb")
            nc.sync.dma_start(
                w2sb[:].rearrange("p (c w) -> p c w", c=8),
                w2t_d.ap().rearrange("(c p) w -> p c w", p=128))
            nc.sync.dma_start(
                w3sb[:].rearrange("p (c w) -> p c w", c=4),
                w3t_d.ap().rearrange("(c p) w -> p c w", p=128))
            nc.sync.dma_start(
                w4sb[:].rearrange("p (c w) -> p c w", c=8),
                w4t_d.ap().rearrange("(c p) w -> p c w", p=128))

            # ---- phase B: bias + ReduceScatter ----
            rs_in = dram.tile([B, 3 * HID], F32, name="rs_in")
            rs_out = dram.tile([R, 3 * HID], F32, name="rs_out")
            for bi, (bs, bsz) in enumerate(cfg.b_tiles):
                so = sb.tile([bsz, 3 * HID], F32, tag="rsin_sb", bufs=2,
                             name=f"so{bi}")
                nc.vector.tensor_add(so[:, 0:HID], pc1[bi][:],
                                     biasc[0:bsz, 0:HID])
                nc.vector.tensor_add(so[:, HID:2 * HID], p1[bi][0][:],
                                     biasc[0:bsz, HID:2 * HID])
                nc.vector.tensor_add(so[:, 2 * HID:3 * HID], p1[bi][1][:],
                                     biasc[0:bsz, 2 * HID:3 * HID])
                nc.sync.dma_start(rs_in[bs:bs + bsz, :], so[:])
            if cfg.no_collective:
                nc.sync.dma_start(rs_out[:], rs_in[0:R, :])
            else:
                nc.gpsimd.collective_compute(
                    "ReduceScatter", ALU.add,
                    replica_groups=[list(range(NC))],
                    ins=[rs_in.opt()], outs=[rs_out.opt()],
                )

            # ---- phase C: replicated load + gate ----
            xall = sb.tile([128, 3 * HID], F32, name="xall")
            for c in range(C):
                nc.sync.dma_start(xall[c * R:(c + 1) * R, :], rs_out[:])
            th = sb.tile([128, HID], F32, name="tanh")
            nc.scalar.activation(th[:], xall[:, 0:HID], ACTF.Tanh)
            ztr = sb.tile([128, HID], F32, name="ztr")
            zr1 = sb.tile([128, 32], F32, name="zr1")
            z = sb.tile([128, 1], F32, name="z")
            nc.vector.tensor_mul(ztr[:], th[:], wc2rep[:])
            nc.vector.reduce_sum(
                zr1[:], ztr[:].rearrange("p (a b) -> p a b", a=32),
                axis=mybir.AxisListType.X)
            nc.vector.reduce_sum(z[:], zr1[:], axis=mybir.AxisListType.X)
            ez = sb.tile([128, 1], F32, name="ez")
            nc.scalar.activation(ez[:], z[:], ACTF.Exp, scale=-1.0)
            ez1 = sb.tile([128, 1], F32, name="ez1")
            nc.vector.tensor_scalar(ez1[:], ez[:], 1.0, None, ALU.add)
            cx = sb.tile([128, 1], F32, name="cx")
            nc.vector.reciprocal(cx[:], ez1[:])
            _pe_keepalive(nc, pst, th[0:1, 0:1], "gate", 0)
            _pe_keepalive(nc, pst, cx[0:1, 0:1], "gate", 1)
            kraw = {}
            for nn_, nm in ((N1, "k1"), (HID, "k2"), (N3, "k3")):
                t = sb.tile([128, 1], F32, name=f"{nm}_raw")
                nc.vector.tensor_scalar(t[:], cx[:], float(nn_), None, ALU.mult)
                kraw[nm] = _floorize(nc, sb, t[:, 0:1], nm)

            do_tail = cfg.phase != "notail"
            if not do_tail:
                nt = sb.tile([R, N3], F32, name="nt")
                nc.vector.tensor_copy(nt[:], xall[0:R, HID:3 * HID])
                nc.vector.tensor_add(nt[:, 0:1], w2sb[0:R, 0:1], w3sb[0:R, 0:1])
                nc.vector.tensor_add(nt[:, 1:2], w4sb[0:R, 0:1], cx[0:R, :])
                nc.vector.tensor_add(nt[:, 2:3], kraw["k1"][0:R, :], kraw["k2"][0:R, :])
                nc.vector.tensor_add(nt[:, 3:4], kraw["k3"][0:R, :], frac[0:R, :])
                nc.vector.tensor_add(nt[:, 4:5], ident[0:R, 0:1], iota16[0:R, 0:1])
                nc.vector.tensor_add(nt[:, 5:6], b2rep[0:R, 0:1], b3rep[0:R, 0:1])
                nc.sync.dma_start(out_d.ap(), nt[:])

            if do_tail:
                # ---- layer 1 kwta + mm2 ----
                x1 = xall[:, HID:3 * HID]
                masked1, thr1 = _kwta(nc, sb, pst, x1, kraw["k1"], N1, consts, "L1")
                xt1 = _transpose_chunks(nc, sb, pst, masked1, N1, ident, True, "L1")
                px2 = ap.tile([128, HID], F32, tag="acc", bufs=6, name="px2")
                w2v = w2sb[:].rearrange("p (c w) -> p c w", c=8)
                for ch in range(8):
                    nc.tensor.matmul(px2[:], xt1[ch][:], w2v[:, ch, :],
                                     start=(ch == 0), stop=(ch == 7))
                x2 = sb.tile([128, HID], F32, name="x2")
                nc.vector.tensor_add(x2[:], px2[:], b2rep[:])

                # ---- layer 2 kwta + mm3 ----
                masked2, thr2 = _kwta(nc, sb, pst, x2[:], kraw["k2"], HID, consts, "L2")
                xt2 = _transpose_chunks(nc, sb, pst, masked2, HID, ident, True, "L2")
                px3 = [ap.tile([128, 512], F32, tag="acc", bufs=6, name=f"px3_{o}") for o in range(2)]
                w3v = w3sb[:].rearrange("p (c w) -> p c w", c=4)
                for ch in range(4):
                    for o in range(2):
                        nc.tensor.matmul(px3[o][:], xt2[ch][:],
                                         w3v[:, ch, 512 * o:512 * (o + 1)],
                                         start=(ch == 0), stop=(ch == 3))
                x3 = sb.tile([128, N3], F32, name="x3")
                nc.vector.tensor_add(x3[:, 0:512], px3[0][:], b3rep[:, 0:512])
                nc.vector.tensor_add(x3[:, 512:1024], px3[1][:], b3rep[:, 512:1024])

                # ---- layer 3 kwta + mm4 ----
                masked3, thr3 = _kwta(nc, sb, pst, x3[:], kraw["k3"], N3, consts, "L3")
                xt3 = _transpose_chunks(nc, sb, pst, masked3, N3, ident, False, "L3")
                px4 = [ap.tile([R, 512], F32, tag="acc", bufs=6, name=f"px4_{o}") for o in range(2)]
                w4v = w4sb[:].rearrange("p (c w) -> p c w", c=8)
                for ch in range(8):
                    for o in range(2):
                        nc.tensor.matmul(px4[o][:], xt3[ch][:],
                                         w4v[:, ch, 512 * o:512 * (o + 1)],
                                         start=(ch == 0), stop=(ch == 7))
                outsb = sb.tile([R, N3], F32, name="outsb")
                nc.vector.tensor_copy(outsb[:, 0:512], px4[0][:])
                nc.vector.tensor_copy(outsb[:, 512:1024], px4[1][:])
                nc.sync.dma_start(out_d.ap(), outsb[:])
                if cfg.debug:
                    nc.sync.dma_start(dbg_rs_d.ap(), xall[0:R, :])
                    gsb = sb.tile([R, 8], F32, name="gsb")
                    nc.vector.tensor_copy(gsb[:, 0:1], cx[0:R, :])
                    nc.vector.tensor_copy(gsb[:, 1:2], kraw["k1"][0:R, :])
                    nc.vector.tensor_copy(gsb[:, 2:3], kraw["k2"][0:R, :])
                    nc.vector.tensor_copy(gsb[:, 3:4], kraw["k3"][0:R, :])
                    nc.vector.tensor_copy(gsb[:, 4:5], thr1[:])
                    nc.vector.tensor_copy(gsb[:, 5:6], thr2[:])
                    nc.vector.tensor_copy(gsb[:, 6:7], thr3[:])
                    nc.vector.tensor_copy(gsb[:, 7:8], z[0:R, :])
                    nc.sync.dma_start(dbg_gate_d.ap(), gsb[:])
                    nc.sync.dma_start(dbg_x2_d.ap(), x2[0:R, :])
                    nc.sync.dma_start(dbg_x3_d.ap(), x3[0:R, :])

    nc.compile()
    return nc


def host_prepare(inputs, cfg: Cfg):
    """Build per-core in_maps from the full inputs."""
    B, NC, KT, SW, KSH = cfg.B, cfg.NC, cfg.KT, cfg.SW, cfg.KSH
    f32 = np.float32
    inp = np.asarray(inputs["input"], f32)
    W_c1 = np.asarray(inputs["W_c1"], f32)
    b_c1 = np.asarray(inputs["b_c1"], f32)
    W_c2 = np.asarray(inputs["W_c2"], f32)
    W1 = np.asarray(inputs["W1"], f32)
    b1 = np.asarray(inputs["b1"], f32)
    W2 = np.asarray(inputs["W2"], f32)
    b2 = np.asarray(inputs["b2"], f32)
    W3 = np.asarray(inputs["W3"], f32)
    b3 = np.asarray(inputs["b3"], f32)
    W4 = np.asarray(inputs["W4"], f32)

    xT = np.ascontiguousarray(inp.T)          # [S2, B]
    wc1T = np.ascontiguousarray(W_c1.T)       # [S2, HID]
    w1T = np.ascontiguousarray(W1.T)          # [S2, N1]

    consts = {
        "ident": np.eye(128, dtype=f32),
        "biasc": np.broadcast_to(
            np.concatenate([b_c1, b1]) / NC, (128, 3 * HID)).copy(),
        "b2rep": np.broadcast_to(b2, (128, HID)).copy(),
        "b3rep": np.broadcast_to(b3, (128, N3)).copy(),
        "wc2rep": np.broadcast_to(W_c2[0], (128, HID)).copy(),
        "frac": ((np.arange(128, dtype=f32) // R + 1.0) / 5.0)[:, None].astype(f32).copy(),
        "iota16": np.broadcast_to(np.arange(16, dtype=f32), (R, 16)).copy(),
        "w2t": np.ascontiguousarray(W2.T),
        "w3t": np.ascontiguousarray(W3.T),
        "w4t": np.ascontiguousarray(W4.T),
    }

    import ml_dtypes
    bf16 = ml_dtypes.bfloat16
    in_maps = []
    for c in range(NC):
        sl = slice(c * KSH, (c + 1) * KSH)
        stream = np.concatenate([xT[sl], wc1T[sl], w1T[sl]], axis=1)  # [KSH, SW]
        hi = stream.astype(bf16)
        lo = (stream - hi.astype(f32)).astype(bf16)
        shl = np.stack([hi, lo], axis=1).reshape(KSH, 2, cfg.SW)  # [KSH,2,SW]
        shl = np.ascontiguousarray(shl.reshape(KT, 128, 2, cfg.SW))
        in_maps.append({"stream": shl, **consts})
    return in_maps


_CACHE = {}


def kernel(**inputs) -> np.ndarray:
    cfg = Cfg(S2=inputs["input"].shape[1], B=inputs["input"].shape[0])
    key = (cfg.S2, cfg.B, cfg.NC)
    if key not in _CACHE:
        _CACHE[key] = build_nc(cfg)
    nc = _CACHE[key]
    in_maps = host_prepare(inputs, cfg)
    res = bass_utils.run_bass_kernel_spmd(
        nc, in_maps, core_ids=list(range(cfg.NC)))
    return np.concatenate([res.results[c]["out"] for c in range(cfg.NC)], axis=0)


if __name__ == "__main__":
    rng = np.random.default_rng(0)
    S2, B = 32768, 256
    demo = {
        "input": rng.standard_normal((B, S2), dtype=np.float32),
        "W_c1": rng.standard_normal((HID, S2), dtype=np.float32) / np.sqrt(S2),
        "b_c1": rng.standard_normal(HID).astype(np.float32) / np.sqrt(S2),
        "W_c2": rng.standard_normal((1, HID), dtype=np.float32) / np.sqrt(HID),
        "W1": rng.standard_normal((N1, S2), dtype=np.float32) / np.sqrt(S2),
        "b1": rng.standard_normal(N1).astype(np.float32) / np.sqrt(S2),
        "W2": rng.standard_normal((HID, N1), dtype=np.float32) / np.sqrt(N1),
        "b2": rng.standard_normal(HID).astype(np.float32) / np.sqrt(N1),
        "W3": rng.standard_normal((N3, HID), dtype=np.float32) / np.sqrt(HID),
        "b3": rng.standard_normal(N3).astype(np.float32) / np.sqrt(HID),
        "W4": rng.standard_normal((N3, N3), dtype=np.float32) / np.sqrt(N3),
    }
    out = kernel(**demo)
    print(out.shape, out.dtype, np.abs(out).max())

